# revision 2
# baseline (speedup 1.0000x reference)
"""Dice-loss kernel for Trainium2 (Bass/Tile), 8-core data-parallel SPMD.

Strategy (v3: packed-index argmax)
----------------------------------
reference: pred = argmax_c(logits); for c in 1..4:
    inter_c = #{v : pred[v]==c and tgt[v]==c},  tsum_c = #{v : tgt[v]==c}
    dice_c = (2*inter_c + eps) / (inter_c + tsum_c + eps); loss = 1 - mean(dice)

The voxel axis (B*D*H*W = 7,077,888) is sharded 8 ways; each core gets
[5, 128, 6912] fp16 logits and [128, 6912] uint16 labels.

Host-side packing: logits are cast to fp16 and the low 3 mantissa bits are
replaced by the class index c: bits = (fp16_bits & ~7) | c.  A float max
over the 5 packed planes then yields the argmax *index* in the low 3 bits
of the winner -- fp16 compare order is unchanged except for values within
8 ULP (~0.4% rel), whose argmax flips are random w.r.t. the independent
labels (measured loss rel-err ~2.5e-4; tolerance 2e-2).

Device per tile (DVE, ~3.75 cyc/voxel total):
  mab  = max(planes 0:2, 2:4)          1 tensor_tensor, 2x mode
  m01  = max(mab0, mab1)               1 tensor_tensor
  m    = max(m01, plane4)              1 tensor_tensor
  pb   = m.bits & 7                    1 tensor_scalar (4x), = pred index
  g    = (pb == tgt)                   1 tensor_tensor, {0,1} fp16
  t_c  = (tgt == c), accum-> tsum_c    4 tensor_scalar (4x) with fused
                                       free-axis sum reduction (op1=add)
PE: inter_c via confusion matmuls accumulated over 128-chunks:
  cm_c += g_chunk^T @ t_c_chunk  (stationary g shared by the 4 classes);
  host takes trace(cm_c) = inter_c.  ACT/DVE stage the PSUM out at the end.

DVE ~28us, DMA ~30us (10.6 MB/core), PE ~13us -> DMA-roofline bound.
"""

import sys
from contextlib import ExitStack

import numpy as np

for _p in ("/opt/trn_rl_repo", "/opt/pypackages"):
    if _p not in sys.path:
        sys.path.append(_p)

import concourse.bacc as bacc
import concourse.bass as bass
import concourse.tile as tile
from concourse import mybir
from concourse.bass_utils import run_bass_kernel_spmd

# Problem shape (hardcoded per contract: kernel.py must be self-contained).
B, C, D, H, W = 2, 5, 96, 192, 192
N_CORES = 8
P = 128                      # SBUF partitions
NVOX = B * D * H * W         # 7,077,888 voxels
SHARD = NVOX // N_CORES      # 884,736 voxels per core
FTOT = SHARD // P            # 6,912 free elems per partition
# Small first tile starts compute sooner; small last tile shortens the
# DVE/PE tail after the final DMA byte.  All multiples of 128 (PE chunks).
TILES = [128, 768, 1536, 1792, 2432, 256]
NT = len(TILES)
NCLS = C - 1                 # foreground classes 1..4
EPS = 1e-8
assert sum(TILES) == FTOT


def emit_dice_kernel(tc, logits_ap, tgt_ap, partials_ap, cms_ap, tiles):
    """Emit the per-core dice partial-sums program into TileContext `tc`.

    logits_ap:   DRAM [C, p, ftot] fp16, class index packed in low 3 bits
    tgt_ap:      DRAM [p, ftot]    uint16 labels 0..4
    partials_ap: DRAM [p, 4*nt]    f32 tsum accum columns, col = c_idx*nt + i
    cms_ap:      DRAM [p, 4*128]   f32 confusion blocks; trace = inter_c
    """
    nc = tc.nc
    nt = len(tiles)
    fdmax = max(tiles)
    fp16 = mybir.dt.float16
    u16 = mybir.dt.uint16
    f32 = mybir.dt.float32
    Alu = mybir.AluOpType
    Act = mybir.ActivationFunctionType
    assert all(fd % 128 == 0 for fd in tiles)

    with ExitStack() as ctx:
        pool_in = ctx.enter_context(tc.tile_pool(name="in", bufs=3))
        pool_t1 = ctx.enter_context(tc.tile_pool(name="t1", bufs=1))
        pool_t2 = ctx.enter_context(tc.tile_pool(name="t2", bufs=2))
        pool_acc = ctx.enter_context(tc.tile_pool(name="acc", bufs=1))
        pool_ps = ctx.enter_context(tc.tile_pool(name="ps", bufs=1, space="PSUM"))

        tsacc = pool_acc.tile([P, 4 * nt], f32, tag="tsacc")
        seven = pool_acc.tile([P, 1], u16, tag="seven")
        nc.vector.memset(seven, 7)
        cm = [
            pool_ps.tile([128, 128], f32, tag=f"cm{q}", name=f"cm{q}")
            for q in range(4)
        ]

        base = 0
        for i, fd in enumerate(tiles):
            sl = slice(base, base + fd)
            base += fd
            first, last = i == 0, i == nt - 1

            lg5 = pool_in.tile([P, 5, fdmax], fp16, tag="lg5")
            tg = pool_in.tile([P, fdmax], u16, tag="tg")
            nc.sync.dma_start(out=tg[:, 0:fd], in_=tgt_ap[:, sl])
            nc.sync.dma_start(
                out=lg5[:, :, 0:fd],
                in_=logits_ap[:, :, sl].rearrange("c p f -> p c f"),
            )

            # max tree over the 5 packed planes (argmax index rides along
            # in the low 3 bits of the winning value)
            mab = pool_t1.tile([P, 2, fdmax], fp16, tag="mab")
            m01 = pool_t1.tile([P, fdmax], fp16, tag="m01")
            m = pool_t1.tile([P, fdmax], fp16, tag="m")
            pb = pool_t1.tile([P, fdmax], u16, tag="pb")
            g = pool_t2.tile([P, fdmax], fp16, tag="g")
            tv = pool_t2.tile([P, 4, fdmax], fp16, tag="tv")
            nc.vector.tensor_tensor(
                mab[:, :, 0:fd], lg5[:, 0:2, 0:fd], lg5[:, 2:4, 0:fd], Alu.max
            )
            nc.vector.tensor_tensor(
                m01[:, 0:fd], mab[:, 0, 0:fd], mab[:, 1, 0:fd], Alu.max
            )
            nc.vector.tensor_tensor(
                m[:, 0:fd], m01[:, 0:fd], lg5[:, 4, 0:fd], Alu.max
            )
            # pred index = winner bits & 7; g = (pred == tgt)
            nc.vector.tensor_scalar(
                pb[:, 0:fd], m.bitcast(u16)[:, 0:fd], seven[:, 0:1], None,
                Alu.bitwise_and,
            )
            nc.vector.tensor_tensor(g[:, 0:fd], pb[:, 0:fd], tg[:, 0:fd], Alu.is_equal)

            # t_c one-hots with fused tsum reduction into per-tile columns
            for c in range(1, 5):
                ci = c - 1
                nc.vector.tensor_scalar(
                    tv[:, ci, 0:fd], tg[:, 0:fd], float(c), None,
                    Alu.is_equal, Alu.add,
                    accum_out=tsacc[:, ci * nt + i : ci * nt + i + 1],
                )
                # inter_c: accumulate g^T t_c on PE (stationary g chunk is
                # shared by the 4 classes; trace(cm_c) = inter_c)
                for k in range(fd // 128):
                    o = k * 128
                    nc.tensor.matmul(
                        cm[ci],
                        g[:, o : o + 128],
                        tv[:, ci, o : o + 128],
                        start=(first and k == 0),
                        stop=(last and k == fd // 128 - 1),
                    )

        nc.sync.dma_start(out=partials_ap, in_=tsacc)
        # PSUM is not DMA-able: stage the confusion blocks through SBUF,
        # split across DVE and ACT (both idle in the tail).
        cmout = pool_acc.tile([P, 4 * 128], f32, tag="cmout")
        for q in range(4):
            dst = cmout[:, q * 128 : (q + 1) * 128]
            if q % 2 == 0:
                nc.vector.tensor_copy(dst, cm[q])
            else:
                nc.scalar.activation(dst, cm[q], Act.Copy)
        nc.sync.dma_start(out=cms_ap, in_=cmout)


_PROGRAM_CACHE = {}


def build_program():
    key = (C, P, FTOT, tuple(TILES))
    if key in _PROGRAM_CACHE:
        return _PROGRAM_CACHE[key]
    nc = bacc.Bacc("TRN2", debug=False, target_bir_lowering=False)
    logits = nc.dram_tensor(
        "logits", [C, P, FTOT], mybir.dt.float16, kind="ExternalInput"
    )
    tgt = nc.dram_tensor("tgt", [P, FTOT], mybir.dt.uint16, kind="ExternalInput")
    partials = nc.dram_tensor(
        "partials", [P, 4 * NT], mybir.dt.float32, kind="ExternalOutput"
    )
    cms = nc.dram_tensor("cms", [P, 4 * 128], mybir.dt.float32, kind="ExternalOutput")
    with tile.TileContext(nc) as tc:
        emit_dice_kernel(
            tc, logits.ap(), tgt.ap(), partials.ap(), cms.ap(), TILES
        )
    nc.compile()
    _PROGRAM_CACHE[key] = nc
    return nc


def make_in_maps(input2, target1):
    lg16 = np.asarray(input2, dtype=np.float32).astype(np.float16)
    u = lg16.view(np.uint16)
    cls = np.arange(C, dtype=np.uint16).reshape(1, C, 1, 1, 1)
    packed = ((u & np.uint16(0xFFF8)) | cls).view(np.float16)
    tg16 = np.asarray(target1).astype(np.uint16)
    lgf = packed.reshape(B, C, NVOX // B)
    tgf = tg16.reshape(B, NVOX // B)
    shards_per_b = N_CORES // B
    s = (NVOX // B) // shards_per_b
    in_maps = []
    for core in range(N_CORES):
        b, q = divmod(core, shards_per_b)
        sl = slice(q * s, (q + 1) * s)
        in_maps.append(
            {
                "logits": np.ascontiguousarray(lgf[b, :, sl]).reshape(C, P, FTOT),
                "tgt": np.ascontiguousarray(tgf[b, sl]).reshape(P, FTOT),
            }
        )
    return in_maps


def _finish(results):
    """Host-side reduction of per-core partials -> scalar loss (float32).

    partials [P, 4*NT]: col c_idx*NT+i = per-partition tsum_c of tile i;
    cms [P, 512]: 4 confusion blocks, trace = inter_c.
    """
    inter = np.zeros(NCLS, dtype=np.float64)
    tsum = np.zeros(NCLS, dtype=np.float64)
    for r in results:
        pa = r["partials"].astype(np.float64).reshape(P, NCLS, NT)
        tsum += pa.sum(axis=(0, 2))
        cms = r["cms"].astype(np.float64)
        for ci in range(NCLS):
            inter[ci] += np.trace(cms[:, ci * 128 : (ci + 1) * 128])
    inter = inter.astype(np.float32)
    tsum = tsum.astype(np.float32)
    eps = np.float32(EPS)
    dice = (np.float32(2.0) * inter + eps) / (inter + tsum + eps)
    loss = np.float32(1.0) - np.mean(dice, dtype=np.float32)
    return np.array([loss], dtype=np.float32)


# test.py can set e.g. RUN_KWARGS.update(trace=True) to profile; the grader
# path leaves this empty.
RUN_KWARGS = {}
LAST_RESULT = None


def kernel(input2, target1):
    global LAST_RESULT
    nc = build_program()
    in_maps = make_in_maps(input2, target1)
    res = run_bass_kernel_spmd(nc, in_maps, core_ids=list(range(N_CORES)), **RUN_KWARGS)
    LAST_RESULT = res
    return _finish(res.results)


# revision 5
# speedup vs baseline: 1.2530x; 1.2530x over previous
"""Dice-loss kernel for Trainium2 (Bass/Tile), 8-core data-parallel SPMD.

Strategy (v3: packed-index argmax)
----------------------------------
reference: pred = argmax_c(logits); for c in 1..4:
    inter_c = #{v : pred[v]==c and tgt[v]==c},  tsum_c = #{v : tgt[v]==c}
    dice_c = (2*inter_c + eps) / (inter_c + tsum_c + eps); loss = 1 - mean(dice)

The voxel axis (B*D*H*W = 7,077,888) is sharded 8 ways; each core gets
[5, 128, 6912] fp16 logits and [128, 6912] uint16 labels.

Host-side packing: logits are cast to fp16 and the low 3 mantissa bits are
replaced by the class index c: bits = (fp16_bits & ~7) | c.  A float max
over the 5 packed planes then yields the argmax *index* in the low 3 bits
of the winner -- fp16 compare order is unchanged except for values within
8 ULP (~0.4% rel), whose argmax flips are random w.r.t. the independent
labels (measured loss rel-err ~2.5e-4; tolerance 2e-2).

Device per tile (DVE, ~3.75 cyc/voxel total):
  mab  = max(planes 0:2, 2:4)          1 tensor_tensor, 2x mode
  m01  = max(mab0, mab1)               1 tensor_tensor
  m    = max(m01, plane4)              1 tensor_tensor
  pb   = m.bits & 7                    1 tensor_scalar (4x), = pred index
  g    = (pb == tgt)                   1 tensor_tensor, {0,1} fp16
  t_c  = (tgt == c), accum-> tsum_c    4 tensor_scalar (4x) with fused
                                       free-axis sum reduction (op1=add)
PE: inter_c via confusion matmuls accumulated over 128-chunks:
  cm_c += g_chunk^T @ t_c_chunk  (stationary g shared by the 4 classes);
  host takes trace(cm_c) = inter_c.  ACT/DVE stage the PSUM out at the end.

DVE ~28us, DMA ~30us (10.6 MB/core), PE ~13us -> DMA-roofline bound.
"""

import sys
from contextlib import ExitStack

import numpy as np

for _p in ("/opt/trn_rl_repo", "/opt/pypackages"):
    if _p not in sys.path:
        sys.path.append(_p)

import concourse.bacc as bacc
import concourse.bass as bass
import concourse.tile as tile
from concourse import mybir
from concourse.bass_utils import run_bass_kernel_spmd

# Problem shape (hardcoded per contract: kernel.py must be self-contained).
B, C, D, H, W = 2, 5, 96, 192, 192
N_CORES = 8
P = 128                      # SBUF partitions
NVOX = B * D * H * W         # 7,077,888 voxels
SHARD = NVOX // N_CORES      # 884,736 voxels per core
FTOT = SHARD // P            # 6,912 free elems per partition
# Small first tile starts compute sooner; small last tile shortens the
# DVE/PE tail after the final DMA byte.  All multiples of 128 (PE chunks).
TILES = [128, 768, 1536, 1792, 2432, 256]
NT = len(TILES)
NCLS = C - 1                 # foreground classes 1..4
EPS = 1e-8
assert sum(TILES) == FTOT


def emit_dice_kernel(tc, logits_ap, tgt_ap, partials_ap, cms_ap, tiles):
    """Emit the per-core dice partial-sums program into TileContext `tc`.

    logits_ap:   DRAM [C, p, ftot] fp16, class index packed in low 3 bits
    tgt_ap:      DRAM [p, ftot]    uint16 labels 0..4
    partials_ap: DRAM [p, 4*nt]    f32 tsum accum columns, col = c_idx*nt + i
    cms_ap:      DRAM [p, 4*128]   f32 confusion blocks; trace = inter_c
    """
    nc = tc.nc
    nt = len(tiles)
    fdmax = max(tiles)
    fp16 = mybir.dt.float16
    u16 = mybir.dt.uint16
    f32 = mybir.dt.float32
    Alu = mybir.AluOpType
    Act = mybir.ActivationFunctionType
    assert all(fd % 128 == 0 for fd in tiles)

    with ExitStack() as ctx:
        pool_in = ctx.enter_context(tc.tile_pool(name="in", bufs=3))
        pool_t1 = ctx.enter_context(tc.tile_pool(name="t1", bufs=1))
        pool_t2 = ctx.enter_context(tc.tile_pool(name="t2", bufs=2))
        pool_acc = ctx.enter_context(tc.tile_pool(name="acc", bufs=1))
        pool_ps = ctx.enter_context(tc.tile_pool(name="ps", bufs=1, space="PSUM"))

        tsacc = pool_acc.tile([P, 4 * nt], f32, tag="tsacc")
        seven = pool_acc.tile([P, 1], u16, tag="seven")
        nc.vector.memset(seven, 7)
        # one PSUM bank [128, 4*128]: class-c block cols c*128..c*128+128,
        # accumulated over all chunks; trace of each block = inter_c
        cm = pool_ps.tile([128, 4 * 128], f32, tag="cm", name="cm")

        base = 0
        for i, fd in enumerate(tiles):
            sl = slice(base, base + fd)
            base += fd
            first, last = i == 0, i == nt - 1

            lg5 = pool_in.tile([P, 5, fdmax], fp16, tag="lg5")
            tg = pool_in.tile([P, fdmax], u16, tag="tg")
            nc.sync.dma_start(out=tg[:, 0:fd], in_=tgt_ap[:, sl])
            nc.sync.dma_start(
                out=lg5[:, :, 0:fd],
                in_=logits_ap[:, :, sl].rearrange("c p f -> p c f"),
            )

            # max tree over the 5 packed planes (argmax index rides along
            # in the low 3 bits of the winning value)
            mab = pool_t1.tile([P, 2, fdmax], fp16, tag="mab")
            m01 = pool_t1.tile([P, fdmax], fp16, tag="m01")
            m = pool_t1.tile([P, fdmax], fp16, tag="m")
            pb = pool_t1.tile([P, fdmax], u16, tag="pb")
            g = pool_t2.tile([P, fdmax], fp16, tag="g")
            tv = pool_t2.tile([P, 4, fdmax], fp16, tag="tv")
            nc.vector.tensor_tensor(
                mab[:, :, 0:fd], lg5[:, 0:2, 0:fd], lg5[:, 2:4, 0:fd], Alu.max
            )
            nc.vector.tensor_tensor(
                m01[:, 0:fd], mab[:, 0, 0:fd], mab[:, 1, 0:fd], Alu.max
            )
            nc.vector.tensor_tensor(
                m[:, 0:fd], m01[:, 0:fd], lg5[:, 4, 0:fd], Alu.max
            )
            # pred index = winner bits & 7; g = (pred == tgt)
            nc.vector.tensor_scalar(
                pb[:, 0:fd], m.bitcast(u16)[:, 0:fd], seven[:, 0:1], None,
                Alu.bitwise_and,
            )
            nc.vector.tensor_tensor(g[:, 0:fd], pb[:, 0:fd], tg[:, 0:fd], Alu.is_equal)

            # t_c one-hots (plain tensor_scalar -> 4x mode; the DVE reduce
            # variant falls back to 1x on HW, so tsum goes to ACT instead)
            dump = pool_t1.tile([P, fdmax], fp16, tag="dump")
            for c in range(1, 5):
                ci = c - 1
                nc.vector.tensor_scalar(
                    tv[:, ci, 0:fd], tg[:, 0:fd], float(c), None, Alu.is_equal
                )
                # tsum_c: free-axis sum on ACT (copy with accumulate)
                nc.scalar.activation(
                    dump[:, 0:fd],
                    tv[:, ci, 0:fd],
                    Act.Copy,
                    accum_out=tsacc[:, ci * nt + i : ci * nt + i + 1],
                )
            # inter_c: one wide matmul per chunk -- stationary g chunk,
            # moving = all 4 t_c chunks; cm block c accumulates g^T t_c
            for k in range(fd // 128):
                o = k * 128
                nc.tensor.matmul(
                    cm,
                    g[:, o : o + 128],
                    tv[:, :, o : o + 128],
                    start=(first and k == 0),
                    stop=(last and k == fd // 128 - 1),
                )

        nc.sync.dma_start(out=partials_ap, in_=tsacc)
        # PSUM is not DMA-able: stage the confusion blocks through SBUF,
        # split across DVE and ACT (both idle in the tail).
        cmout = pool_acc.tile([P, 4 * 128], f32, tag="cmout")
        nc.vector.tensor_copy(cmout[:, 0:256], cm[:, 0:256])
        nc.scalar.activation(cmout[:, 256:512], cm[:, 256:512], Act.Copy)
        nc.sync.dma_start(out=cms_ap, in_=cmout)


_PROGRAM_CACHE = {}


def build_program():
    key = (C, P, FTOT, tuple(TILES))
    if key in _PROGRAM_CACHE:
        return _PROGRAM_CACHE[key]
    nc = bacc.Bacc("TRN2", debug=False, target_bir_lowering=False)
    logits = nc.dram_tensor(
        "logits", [C, P, FTOT], mybir.dt.float16, kind="ExternalInput"
    )
    tgt = nc.dram_tensor("tgt", [P, FTOT], mybir.dt.uint16, kind="ExternalInput")
    partials = nc.dram_tensor(
        "partials", [P, 4 * NT], mybir.dt.float32, kind="ExternalOutput"
    )
    cms = nc.dram_tensor("cms", [P, 4 * 128], mybir.dt.float32, kind="ExternalOutput")
    with tile.TileContext(nc) as tc:
        emit_dice_kernel(
            tc, logits.ap(), tgt.ap(), partials.ap(), cms.ap(), TILES
        )
    nc.compile()
    _PROGRAM_CACHE[key] = nc
    return nc


def make_in_maps(input2, target1):
    lg16 = np.asarray(input2, dtype=np.float32).astype(np.float16)
    u = lg16.view(np.uint16)
    cls = np.arange(C, dtype=np.uint16).reshape(1, C, 1, 1, 1)
    packed = ((u & np.uint16(0xFFF8)) | cls).view(np.float16)
    tg16 = np.asarray(target1).astype(np.uint16)
    lgf = packed.reshape(B, C, NVOX // B)
    tgf = tg16.reshape(B, NVOX // B)
    shards_per_b = N_CORES // B
    s = (NVOX // B) // shards_per_b
    in_maps = []
    for core in range(N_CORES):
        b, q = divmod(core, shards_per_b)
        sl = slice(q * s, (q + 1) * s)
        in_maps.append(
            {
                "logits": np.ascontiguousarray(lgf[b, :, sl]).reshape(C, P, FTOT),
                "tgt": np.ascontiguousarray(tgf[b, sl]).reshape(P, FTOT),
            }
        )
    return in_maps


def _finish(results):
    """Host-side reduction of per-core partials -> scalar loss (float32).

    partials [P, 4*NT]: col c_idx*NT+i = per-partition tsum_c of tile i;
    cms [P, 512]: 4 confusion blocks, trace = inter_c.
    """
    inter = np.zeros(NCLS, dtype=np.float64)
    tsum = np.zeros(NCLS, dtype=np.float64)
    for r in results:
        pa = r["partials"].astype(np.float64).reshape(P, NCLS, NT)
        tsum += pa.sum(axis=(0, 2))
        cms = r["cms"].astype(np.float64)
        for ci in range(NCLS):
            inter[ci] += np.trace(cms[:, ci * 128 : (ci + 1) * 128])
    inter = inter.astype(np.float32)
    tsum = tsum.astype(np.float32)
    eps = np.float32(EPS)
    dice = (np.float32(2.0) * inter + eps) / (inter + tsum + eps)
    loss = np.float32(1.0) - np.mean(dice, dtype=np.float32)
    return np.array([loss], dtype=np.float32)


# test.py can set e.g. RUN_KWARGS.update(trace=True) to profile; the grader
# path leaves this empty.
RUN_KWARGS = {}
LAST_RESULT = None


def kernel(input2, target1):
    global LAST_RESULT
    nc = build_program()
    in_maps = make_in_maps(input2, target1)
    res = run_bass_kernel_spmd(nc, in_maps, core_ids=list(range(N_CORES)), **RUN_KWARGS)
    LAST_RESULT = res
    return _finish(res.results)


# revision 12
# speedup vs baseline: 1.3016x; 1.0388x over previous
"""Dice-loss kernel for Trainium2 (Bass/Tile), 8-core data-parallel SPMD.

Strategy (v3: packed-index argmax)
----------------------------------
reference: pred = argmax_c(logits); for c in 1..4:
    inter_c = #{v : pred[v]==c and tgt[v]==c},  tsum_c = #{v : tgt[v]==c}
    dice_c = (2*inter_c + eps) / (inter_c + tsum_c + eps); loss = 1 - mean(dice)

The voxel axis (B*D*H*W = 7,077,888) is sharded 8 ways; each core gets
[5, 128, 6912] fp16 logits and [128, 6912] uint16 labels.

Host-side packing: logits are cast to fp16 and the low 3 mantissa bits are
replaced by the class index c: bits = (fp16_bits & ~7) | c.  A float max
over the 5 packed planes then yields the argmax *index* in the low 3 bits
of the winner -- fp16 compare order is unchanged except for values within
8 ULP (~0.4% rel), whose argmax flips are random w.r.t. the independent
labels (measured loss rel-err ~2.5e-4; tolerance 2e-2).

Device per tile (DVE, ~3.75 cyc/voxel total):
  mab  = max(planes 0:2, 2:4)          1 tensor_tensor, 2x mode
  m01  = max(mab0, mab1)               1 tensor_tensor
  m    = max(m01, plane4)              1 tensor_tensor
  pb   = m.bits & 7                    1 tensor_scalar (4x), = pred index
  g    = (pb == tgt)                   1 tensor_tensor, {0,1} fp16
  t_c  = (tgt == c), accum-> tsum_c    4 tensor_scalar (4x) with fused
                                       free-axis sum reduction (op1=add)
PE: inter_c via confusion matmuls accumulated over 128-chunks:
  cm_c += g_chunk^T @ t_c_chunk  (stationary g shared by the 4 classes);
  host takes trace(cm_c) = inter_c.  ACT/DVE stage the PSUM out at the end.

DVE ~28us, DMA ~30us (10.6 MB/core), PE ~13us -> DMA-roofline bound.
"""

import sys
from contextlib import ExitStack

import numpy as np

for _p in ("/opt/trn_rl_repo", "/opt/pypackages"):
    if _p not in sys.path:
        sys.path.append(_p)

import concourse.bacc as bacc
import concourse.bass as bass
import concourse.tile as tile
from concourse import mybir
from concourse.bass_utils import run_bass_kernel_spmd

# Problem shape (hardcoded per contract: kernel.py must be self-contained).
B, C, D, H, W = 2, 5, 96, 192, 192
N_CORES = 8
P = 128                      # SBUF partitions
NVOX = B * D * H * W         # 7,077,888 voxels
SHARD = NVOX // N_CORES      # 884,736 voxels per core
FTOT = SHARD // P            # 6,912 free elems per partition
# Small first tile starts compute sooner; big tiles next (amortize per-op
# overheads while DMA streams ahead); small last tiles shorten the DVE/PE
# drain after the final DMA byte.  All multiples of 128 (PE chunks).
TILES = [128, 2432, 2176, 1536, 512, 128]
NT = len(TILES)
NCLS = C - 1                 # foreground classes 1..4
EPS = 1e-8
assert sum(TILES) == FTOT


def emit_dice_kernel(tc, logits_ap, tgt_ap, partials_ap, cms_ap, ts34_ap, tiles):
    """Emit the per-core dice partial-sums program into TileContext `tc`.

    logits_ap:   DRAM [C, p, ftot] fp16, class index packed in low 3 bits
    tgt_ap:      DRAM [p, ftot]    uint16 labels 0..4
    partials_ap: DRAM [p, 2*nt+2]  f32 ACT tsum accum columns:
                 tsum_1 tile i -> col i, tsum_2 tile i -> col nt+i,
                 tsum_3/tsum_4 of tile 0 -> cols 2nt, 2nt+1
    cms_ap:      DRAM [p, 4*128]   f32 confusion blocks; trace = inter_c
    ts34_ap:     DRAM [1, 1024]    f32 PE tsum_3 (cols 0:512) / tsum_4 rows
    """
    nc = tc.nc
    nt = len(tiles)
    fdmax = max(tiles)
    fp16 = mybir.dt.float16
    u16 = mybir.dt.uint16
    f32 = mybir.dt.float32
    Alu = mybir.AluOpType
    Act = mybir.ActivationFunctionType
    assert all(fd % 128 == 0 for fd in tiles)

    with ExitStack() as ctx:
        pool_in = ctx.enter_context(tc.tile_pool(name="in", bufs=1))
        pool_t1 = ctx.enter_context(tc.tile_pool(name="t1", bufs=1))
        pool_t2 = ctx.enter_context(tc.tile_pool(name="t2", bufs=2))
        pool_acc = ctx.enter_context(tc.tile_pool(name="acc", bufs=1))
        pool_ps = ctx.enter_context(tc.tile_pool(name="ps", bufs=1, space="PSUM"))

        tsacc = pool_acc.tile([P, 2 * nt + 2], f32, tag="tsacc")
        seven = pool_acc.tile([P, 1], u16, tag="seven")
        ones = pool_acc.tile([P, 1], fp16, tag="ones")
        nc.vector.memset(seven, 7)
        nc.vector.memset(ones, 1.0)
        # one PSUM bank [128, 4*128]: class-c block cols c*128..c*128+128,
        # accumulated over all chunks; trace of each block = inter_c
        cm = pool_ps.tile([128, 4 * 128], f32, tag="cm", name="cm")
        # tsum_3/tsum_4 partial rows via PE ones-matmuls (tiles 1..nt-1);
        # tile 0 (128 wide) goes to ACT so the start=True matmul covers the
        # full 512 columns (PSUM zero rule).
        ts34 = [
            pool_ps.tile([1, 512], f32, tag=f"ts34_{q}", name=f"ts34_{q}")
            for q in range(2)
        ]

        # Every tile owns its exact-size input buffers (no pool recycling):
        # all input DMAs are issued up-front and drain FIFO at full HBM rate.
        in_tiles = []
        base = 0
        for i, fd in enumerate(tiles):
            sl = slice(base, base + fd)
            base += fd
            lg5 = pool_in.tile([P, 5, fd], fp16, tag=f"lg5_{i}")
            tg = pool_in.tile([P, fd], u16, tag=f"tg_{i}")
            nc.sync.dma_start(out=tg, in_=tgt_ap[:, sl])
            nc.sync.dma_start(
                out=lg5, in_=logits_ap[:, :, sl].rearrange("c p f -> p c f")
            )
            in_tiles.append((lg5, tg))

        def chunk_list(fd, w):
            out, off = [], 0
            while off < fd:
                ww = min(w, fd - off)
                out.append((off, ww))
                off += ww
            return out

        for i, fd in enumerate(tiles):
            lg5, tg = in_tiles[i]
            first, last = i == 0, i == nt - 1

            # t_c one-hots first: they only need tg (arrives before the big
            # lg5 transfer), so DVE starts while lg5 is still streaming.
            g = pool_t2.tile([P, fdmax], fp16, tag="g")
            tv = pool_t2.tile([P, 4, fdmax], fp16, tag="tv")
            for c in range(1, 5):
                ci = c - 1
                nc.vector.tensor_scalar(
                    tv[:, ci, 0:fd], tg, float(c), None, Alu.is_equal
                )

            # max tree over the 5 packed planes (argmax index rides along
            # in the low 3 bits of the winning value)
            mab = pool_t1.tile([P, 2, fdmax], fp16, tag="mab")
            m01 = pool_t1.tile([P, fdmax], fp16, tag="m01")
            m = pool_t1.tile([P, fdmax], fp16, tag="m")
            pb = pool_t1.tile([P, fdmax], u16, tag="pb")
            nc.vector.tensor_tensor(mab[:, :, 0:fd], lg5[:, 0:2], lg5[:, 2:4], Alu.max)
            nc.vector.tensor_tensor(
                m01[:, 0:fd], mab[:, 0, 0:fd], mab[:, 1, 0:fd], Alu.max
            )
            nc.vector.tensor_tensor(m[:, 0:fd], m01[:, 0:fd], lg5[:, 4], Alu.max)
            # pred index = winner bits & 7; g = (pred == tgt)
            nc.vector.tensor_scalar(
                pb[:, 0:fd], m.bitcast(u16)[:, 0:fd], seven[:, 0:1], None,
                Alu.bitwise_and,
            )
            nc.vector.tensor_tensor(g[:, 0:fd], pb[:, 0:fd], tg, Alu.is_equal)

            # tsum_1/2 on ACT (copy with accumulate); tsum_3/4 on PE below
            # (tile 0's tsum_3/4 also on ACT, see PSUM zero rule above)
            dump = pool_t1.tile([P, fdmax], fp16, tag="dump")
            act_cls = (0, 1, 2, 3) if first else (0, 1)
            for ci in act_cls:
                col = ci * nt + i if ci < 2 else 2 * nt + (ci - 2)
                nc.scalar.activation(
                    dump[:, 0:fd],
                    tv[:, ci, 0:fd],
                    Act.Copy,
                    accum_out=tsacc[:, col : col + 1],
                )
            # inter_c: one wide matmul per chunk -- stationary g chunk,
            # moving = all 4 t_c chunks; cm block c accumulates g^T t_c
            for k in range(fd // 128):
                o = k * 128
                nc.tensor.matmul(
                    cm,
                    g[:, o : o + 128],
                    tv[:, :, o : o + 128],
                    start=(first and k == 0),
                    stop=(last and k == fd // 128 - 1),
                )
            if not first:
                for q in range(2):
                    for o, w in chunk_list(fd, 512):
                        nc.tensor.matmul(
                            ts34[q][:, 0:w],
                            ones,
                            tv[:, 2 + q, o : o + w],
                            start=(i == 1 and o == 0),
                            stop=(last and o + w == fd),
                        )

        nc.sync.dma_start(out=partials_ap, in_=tsacc)
        # PSUM is not DMA-able: stage the confusion blocks and the ts34 rows
        # through SBUF, split across DVE and ACT (both idle in the tail).
        cmout = pool_acc.tile([P, 4 * 128], f32, tag="cmout")
        ts34out = pool_acc.tile([1, 1024], f32, tag="ts34out")
        nc.vector.tensor_copy(cmout[:, 0:256], cm[:, 0:256])
        nc.scalar.activation(cmout[:, 256:512], cm[:, 256:512], Act.Copy)
        nc.vector.tensor_copy(ts34out[:, 0:512], ts34[0])
        nc.scalar.activation(ts34out[:, 512:1024], ts34[1], Act.Copy)
        nc.sync.dma_start(out=cms_ap, in_=cmout)
        nc.sync.dma_start(out=ts34_ap, in_=ts34out)


_PROGRAM_CACHE = {}


def build_program():
    key = (C, P, FTOT, tuple(TILES))
    if key in _PROGRAM_CACHE:
        return _PROGRAM_CACHE[key]
    nc = bacc.Bacc("TRN2", debug=False, target_bir_lowering=False)
    logits = nc.dram_tensor(
        "logits", [C, P, FTOT], mybir.dt.float16, kind="ExternalInput"
    )
    tgt = nc.dram_tensor("tgt", [P, FTOT], mybir.dt.uint16, kind="ExternalInput")
    partials = nc.dram_tensor(
        "partials", [P, 2 * NT + 2], mybir.dt.float32, kind="ExternalOutput"
    )
    cms = nc.dram_tensor("cms", [P, 4 * 128], mybir.dt.float32, kind="ExternalOutput")
    ts34 = nc.dram_tensor("ts34", [1, 1024], mybir.dt.float32, kind="ExternalOutput")
    with tile.TileContext(nc) as tc:
        emit_dice_kernel(
            tc, logits.ap(), tgt.ap(), partials.ap(), cms.ap(), ts34.ap(), TILES
        )
    nc.compile()
    _PROGRAM_CACHE[key] = nc
    return nc


def make_in_maps(input2, target1):
    lg16 = np.asarray(input2, dtype=np.float32).astype(np.float16)
    u = lg16.view(np.uint16)
    cls = np.arange(C, dtype=np.uint16).reshape(1, C, 1, 1, 1)
    packed = ((u & np.uint16(0xFFF8)) | cls).view(np.float16)
    tg16 = np.asarray(target1).astype(np.uint16)
    lgf = packed.reshape(B, C, NVOX // B)
    tgf = tg16.reshape(B, NVOX // B)
    shards_per_b = N_CORES // B
    s = (NVOX // B) // shards_per_b
    in_maps = []
    for core in range(N_CORES):
        b, q = divmod(core, shards_per_b)
        sl = slice(q * s, (q + 1) * s)
        in_maps.append(
            {
                "logits": np.ascontiguousarray(lgf[b, :, sl]).reshape(C, P, FTOT),
                "tgt": np.ascontiguousarray(tgf[b, sl]).reshape(P, FTOT),
            }
        )
    return in_maps


def _finish(results):
    """Host-side reduction of per-core partials -> scalar loss (float32).

    partials [P, 2*NT+2]: ACT accum cols (tsum_1 tiles, tsum_2 tiles,
    tile-0 tsum_3, tile-0 tsum_4); ts34 [1, 1024]: PE tsum_3/tsum_4 rows;
    cms [P, 512]: 4 confusion blocks, trace = inter_c.
    """
    inter = np.zeros(NCLS, dtype=np.float64)
    tsum = np.zeros(NCLS, dtype=np.float64)
    for r in results:
        pa = r["partials"].astype(np.float64)
        ts34 = r["ts34"].astype(np.float64).reshape(2, 512)
        tsum[0] += pa[:, 0:NT].sum()
        tsum[1] += pa[:, NT : 2 * NT].sum()
        tsum[2] += pa[:, 2 * NT].sum() + ts34[0].sum()
        tsum[3] += pa[:, 2 * NT + 1].sum() + ts34[1].sum()
        cms = r["cms"].astype(np.float64)
        for ci in range(NCLS):
            inter[ci] += np.trace(cms[:, ci * 128 : (ci + 1) * 128])
    inter = inter.astype(np.float32)
    tsum = tsum.astype(np.float32)
    eps = np.float32(EPS)
    dice = (np.float32(2.0) * inter + eps) / (inter + tsum + eps)
    loss = np.float32(1.0) - np.mean(dice, dtype=np.float32)
    return np.array([loss], dtype=np.float32)


# test.py can set e.g. RUN_KWARGS.update(trace=True) to profile; the grader
# path leaves this empty.
RUN_KWARGS = {}
LAST_RESULT = None


def kernel(input2, target1):
    global LAST_RESULT
    nc = build_program()
    in_maps = make_in_maps(input2, target1)
    res = run_bass_kernel_spmd(nc, in_maps, core_ids=list(range(N_CORES)), **RUN_KWARGS)
    LAST_RESULT = res
    return _finish(res.results)
